# revision 1
# baseline (speedup 1.0000x reference)
"""NetVLAD pooling kernel for Trainium2 (Bass/Tile), SPMD over 8 NeuronCores.

Reference computation (per sample n):
    x_hat = x / ||x||_C                      # L2 norm over channels, per position
    logits = fc_w @ x_hat + fc_b             # [K, S]
    soft = softmax_K(logits)                 # [K, S]
    a_sum = soft.sum(S)                      # [K]
    vlad = soft @ x_hat^T - a_sum[:,None] * centroids     # [K, C]
    vlad = intra_l2norm(vlad) ; flatten ; global l2norm

Kernel strategy (per core, data-parallel over N):
  Layout A: x tiles [C=128 partitions, s free].  Layout B: [s=128 partitions, ...].
  Per 128-position tile:
    mm1 (PE):  stationary = x tile [c,128]; moving = [I | fc_w^T] (192/256 rows)
               -> PSUM [s, 0:128]=x^T, [s,128:192]=raw logits^T  (layout B)
    ssq (PE):  stationary = x^2_f16 tile; moving = ones column -> PSUM ssq [s, 1]
  Per group of TPG tiles, layout-B elementwise:
    norm=sqrt(ssq) (ACT), rnorm=1/norm (DVE)
    t = raw_logitsT * rnorm + bias_bcast  (DVE, f16)
    E = exp(t) (ACT, f16);  sumexp = reduce_K(E) (DVE); rsum = 1/sumexp
    w^T = E * (rsum*rnorm)  (DVE, f16)          # w = soft * rnorm
    x^T copy PSUM->SBUF f16 (split DVE/ACT), norm col appended
    mm2 (PE): stationary = w^T [s,64]; moving = [x^T | norm] (129 rows)
              -> PSUM accum vlad[k, 0:128], a_sum[k, 128]   (a_sum = sum_s w*norm)
  Epilogue per sample: vlad - a_sum*centroids, intra-norm, global norm = /8 folded.

The final global L2 norm of the intra-normalized VLAD is sqrt(K)=8 exactly
(each of the 64 rows is unit-norm), so it is folded as a constant 0.125.
"""

import contextlib
import numpy as np

import concourse.bacc as bacc
import concourse.bass as bass
import concourse.mybir as mybir
import concourse.tile as tile

N, C, S, K = 16, 128, 16384, 64
N_CORES = 8
N_PER_CORE = N // N_CORES  # 2

F32 = mybir.dt.float32
F32R = mybir.dt.float32r
F16 = mybir.dt.float16
AF = mybir.ActivationFunctionType
ALU = mybir.AluOpType
AX = mybir.AxisListType

TILE = 128           # positions per matmul tile

# tuning knobs (overridable via build_nc(opts=...)); defaults = best HW-tuned
DEFAULT_OPTS = dict(
    xt_dve_cols=32,     # share of x^T PSUM->SBUF copy on DVE (rest ACT)
    x2_engine="dve",    # "dve" | "gpsimd" | "act": who squares x for the ssq matmul
    fold_q=False,       # scale x^T by q=rsum*rnorm during the PSUM copy; mm2
                        # stationary becomes E directly (no w^T op)
    fp32r=False,        # feed PE fp32r directly (no f16 cast of x)
    group=512,          # positions per elementwise group
    mm1_bufs=3,         # PSUM depth for mm1 output (3 x 2 banks)
    reduce_split=False, # per-block 2D reduces instead of one 3D reduce
    ssq_bufs=1,
    sbuf_bufs=2,        # extra depth added to the sbuf working pools
    ablate=(),          # timing ablations: subset of {"mm1","mm2","softmax_prep",
                        #   "reduce","xtcopy","cast","x2","exp","dma"}
)


def build_nc(n_samples=N_PER_CORE, s_len=S, finalize=True, repeat=1, opts=None):
    """Build the Bass module for one core processing `n_samples` samples.

    `repeat` re-runs the whole computation that many times on-device inside a
    dynamic loop (benchmarking only).
    """
    o = dict(DEFAULT_OPTS)
    if opts:
        o.update(opts)
    group = o["group"]
    tpg = group // TILE
    mm1w = 256 if o["fp32r"] else 192  # moving width of mm1

    nc = bacc.Bacc("TRN2", target_bir_lowering=False, debug=False)

    x_d = nc.dram_tensor("x", [n_samples, C, s_len], F32, kind="ExternalInput")
    fcw_d = nc.dram_tensor("fc_w", [K, C], F32, kind="ExternalInput")
    fcb_d = nc.dram_tensor("fc_b", [1, K], F32, kind="ExternalInput")
    cent_d = nc.dram_tensor("centroids", [K, C], F32, kind="ExternalInput")
    out_d = nc.dram_tensor("out", [n_samples, K, C], F32, kind="ExternalOutput")

    n_groups = s_len // group

    with tile.TileContext(nc) as tc:
        with (
            tc.tile_pool(name="const", bufs=1) as const_pool,
            tc.tile_pool(name="xf", bufs=3 + o["sbuf_bufs"]) as x_pool,
            tc.tile_pool(name="xh", bufs=3 + o["sbuf_bufs"]) as xh_pool,
            tc.tile_pool(name="x2", bufs=2 + o["sbuf_bufs"]) as x2_pool,
            tc.tile_pool(name="xt", bufs=2 + o["sbuf_bufs"]) as xt_pool,
            tc.tile_pool(name="ew", bufs=2 + o["sbuf_bufs"]) as ew_pool,
            tc.tile_pool(name="sm", bufs=3 + o["sbuf_bufs"]) as sm_pool,
            tc.tile_pool(name="ep", bufs=1) as ep_pool,
        ):
            # ---------------- constants ----------------
            ones_f32 = const_pool.tile([128, 128], F32, tag="ones_f32")
            nc.vector.memset(ones_f32[:], 1.0)
            ident_f32 = const_pool.tile([128, 128], F32, tag="ident_f32")
            # iota = f*1 + p*(-1); select in_ where ==0 else fill
            nc.gpsimd.affine_select(
                ident_f32[:], ones_f32[:], pattern=[[1, 128]],
                compare_op=ALU.is_equal, fill=0.0, base=0, channel_multiplier=-1,
            )
            ones_col_f16 = const_pool.tile([128, 1], F16, tag="ones_col")
            nc.vector.memset(ones_col_f16[:], 1.0)

            fcw_sb = const_pool.tile([K, C], F32, tag="fcw")
            nc.sync.dma_start(out=fcw_sb[:], in_=fcw_d.ap())
            fcb_sb = const_pool.tile([1, K], F32, tag="fcb")
            nc.sync.dma_start(out=fcb_sb[:], in_=fcb_d.ap())
            cent_sb = const_pool.tile([K, C], F32, tag="cent")
            nc.sync.dma_start(out=cent_sb[:], in_=cent_d.ap())

            # rhs_const = [I | fc_w^T (| pad)] in f16, or f32 when fp32r
            rdt = F32 if o["fp32r"] else F16
            rhs_const = const_pool.tile([128, mm1w], rdt, tag="rhs_const")
            bias_bcast = const_pool.tile([128, K], F16, tag="bias_bcast")
            bias4 = const_pool.tile([128, tpg * K], F16, tag="bias4")
            if o["fp32r"] and mm1w > 192:
                nc.vector.memset(rhs_const[:, 192:mm1w], 0.0)
            nc.vector.tensor_copy(rhs_const[:, 0:128], ident_f32[:])
            with tc.tile_pool(name="ipsum", bufs=1, space="PSUM") as ipsum_pool:
                fcwT_psum = ipsum_pool.tile([128, K], F32, tag="init")
                nc.tensor.transpose(fcwT_psum[:], fcw_sb[:], ident_f32[0:K, 0:K])
                nc.vector.tensor_copy(rhs_const[:, 128:192], fcwT_psum[:])

                # bias_bcast[s, k] = fc_b[k]  (outer product ones x fc_b)
                bias_psum = ipsum_pool.tile([128, K], F32, tag="init")
                nc.tensor.matmul(
                    bias_psum[:], lhsT=ones_f32[0:1, :], rhs=fcb_sb[:],
                    start=True, stop=True, skip_group_check=True,
                )
                nc.vector.tensor_copy(bias_bcast[:], bias_psum[:])
                b4_3d = bias4[:].rearrange("p (t x) -> p t x", t=tpg)
                nc.vector.tensor_copy(
                    b4_3d, bias_psum[:].unsqueeze(1).broadcast_to((128, tpg, K)))

            with (
                tc.tile_pool(name="mm1", bufs=o["mm1_bufs"], space="PSUM") as mm1_pool,
                tc.tile_pool(name="ssqp", bufs=o["ssq_bufs"], space="PSUM") as ssq_pool,
                tc.tile_pool(name="vladp", bufs=1, space="PSUM") as vlad_pool,
            ):
                env = dict(
                    o=o, group=group, tpg=tpg, mm1w=mm1w, n_groups=n_groups,
                    n_samples=n_samples,
                    x_pool=x_pool, xh_pool=xh_pool, x2_pool=x2_pool,
                    mm1_pool=mm1_pool, ssq_pool=ssq_pool, vlad_pool=vlad_pool,
                    xt_pool=xt_pool, ew_pool=ew_pool, sm_pool=sm_pool,
                    ep_pool=ep_pool,
                    rhs_const=rhs_const, bias_bcast=bias_bcast, bias4=bias4,
                    ones_col_f16=ones_col_f16, cent_sb=cent_sb,
                )

                loop_ctx = (tc.For_i(0, repeat, 1) if repeat > 1
                            else contextlib.nullcontext())
                with loop_ctx:
                    _main_body(nc, x_d.ap(), out_d.ap(), env)

    if finalize:
        nc.finalize()
    return nc


def _main_body(nc, x_ap, out_ap, env):
    o = env["o"]
    group, tpg, mm1w = env["group"], env["tpg"], env["mm1w"]
    n_samples, n_groups = env["n_samples"], env["n_groups"]
    x_pool = env["x_pool"]; xh_pool = env["xh_pool"]; x2_pool = env["x2_pool"]
    mm1_pool = env["mm1_pool"]; ssq_pool = env["ssq_pool"]
    vlad_pool = env["vlad_pool"]; xt_pool = env["xt_pool"]
    ew_pool = env["ew_pool"]; sm_pool = env["sm_pool"]; ep_pool = env["ep_pool"]
    rhs_const = env["rhs_const"]; bias_bcast = env["bias_bcast"]
    bias4 = env["bias4"]
    ones_col_f16 = env["ones_col_f16"]; cent_sb = env["cent_sb"]
    xt_dve = o["xt_dve_cols"]
    ab = set(o.get("ablate") or ())

    for n in range(n_samples):
        vlad_psum = vlad_pool.tile([K, 132], F32)
        for g in range(n_groups):
            first_g = g == 0
            xf = x_pool.tile([128, group], F32)
            if "dma" in ab:
                nc.sync.dma_start(out=xf[:, 0:4], in_=x_ap[n][:, 0:4])
            else:
                nc.sync.dma_start(out=xf[:], in_=x_ap[n][:, g * group:(g + 1) * group])
            if o["fp32r"]:
                xh = xf  # PE consumes fp32r view; no cast op
                xsrc_f16 = None
            else:
                xh = xh_pool.tile([128, group], F16)
                if "cast" not in ab or first_g:
                    nc.gpsimd.tensor_copy(xh[:], xf[:])
                xsrc_f16 = xh
            x2 = x2_pool.tile([128, group], F16)
            x2src = xf if o["fp32r"] else xsrc_f16
            if "x2" not in ab or first_g:
                if o["x2_engine"] == "dve":
                    nc.vector.tensor_mul(x2[:], x2src[:], x2src[:])
                elif o["x2_engine"] == "act":
                    nc.scalar.square(x2[:], x2src[:])
                else:
                    nc.gpsimd.tensor_mul(x2[:], x2src[:], x2src[:])

            mm1p = mm1_pool.tile([128, 256 * tpg], F32)
            ssqp = ssq_pool.tile([128, tpg], F32)
            mm1_tiles = range(0) if ("mm1" in ab and not first_g) else range(tpg)
            for t in mm1_tiles:
                lhs = xh[:, t * TILE:(t + 1) * TILE]
                rhs = rhs_const[:]
                if o["fp32r"]:
                    lhs = lhs.bitcast(F32R)
                    rhs = rhs.bitcast(F32R)
                nc.tensor.matmul(
                    mm1p[:, t * 256: t * 256 + mm1w],
                    lhsT=lhs, rhs=rhs,
                    start=True, stop=True, skip_group_check=True,
                )
                nc.tensor.matmul(
                    ssqp[:, t:t + 1],
                    lhsT=x2[:, t * TILE:(t + 1) * TILE],
                    rhs=ones_col_f16[:],
                    start=True, stop=True, skip_group_check=True,
                )

            mm1_3d = mm1p[:].rearrange("p (t x) -> p t x", t=tpg)
            logits_3d = mm1_3d[:, :, 128:192]

            normv = sm_pool.tile([128, tpg], F32, tag="normv")
            nc.scalar.activation(normv[:], ssqp[:], func=AF.Sqrt)
            rnorm = sm_pool.tile([128, tpg], F32, tag="rnorm")
            nc.vector.reciprocal(rnorm[:], normv[:])
            rnorm_b = rnorm[:].unsqueeze(-1).broadcast_to((128, tpg, K))

            # t2 = raw*rnorm + bias
            t2 = ew_pool.tile([128, tpg * K], F16, tag="t2")
            t2_3d = t2[:].rearrange("p (t x) -> p t x", t=tpg)
            if "softmax_prep" not in ab:
                t1 = ew_pool.tile([128, tpg * K], F16, tag="t1")
                t1_3d = t1[:].rearrange("p (t x) -> p t x", t=tpg)
                nc.vector.tensor_mul(t1_3d, logits_3d, rnorm_b)
                nc.vector.tensor_add(t2[:], t1[:], bias4[:])
                e_in = t2[:]
            else:
                e_in = t2[:] if first_g else t2[:]
                if first_g:
                    nc.vector.tensor_mul(t2_3d, logits_3d, rnorm_b)

            # E = exp(t2); sumexp; rsum; q
            E = ew_pool.tile([128, tpg * K], F16, tag="E")
            if "exp" not in ab or first_g:
                nc.scalar.activation(E[:], e_in, func=AF.Exp)
            E_3d = E[:].rearrange("p (t x) -> p t x", t=tpg)
            sume = sm_pool.tile([128, tpg], F32, tag="sume")
            if o["reduce_split"]:
                for t in range(tpg):
                    nc.vector.tensor_reduce(
                        sume[:, t:t + 1], E[:, t * K:(t + 1) * K],
                        axis=AX.X, op=ALU.add)
            else:
                nc.vector.tensor_reduce(sume[:], E_3d, axis=AX.X, op=ALU.add)
            rsum = sm_pool.tile([128, tpg], F32, tag="rsum")
            nc.vector.reciprocal(rsum[:], sume[:])
            qsc = sm_pool.tile([128, tpg], F32, tag="qsc")
            nc.vector.tensor_mul(qsc[:], rsum[:], rnorm[:])
            if o["fold_q"]:
                wt = E
            else:
                wt = ew_pool.tile([128, tpg * K], F16, tag="wt")
                wt_3d = wt[:].rearrange("p (t x) -> p t x", t=tpg)
                q_b = qsc[:].unsqueeze(-1).broadcast_to((128, tpg, K))
                nc.vector.tensor_mul(wt_3d, E_3d, q_b)

            # x^T copy (+ norm/rsum col) into [128, tpg*132] f16
            xt = xt_pool.tile([128, tpg * 132], F16)
            xt_3d = xt[:].rearrange("p (t x) -> p t x", t=tpg)
            if "xtcopy" not in ab or first_g:
                if o["fold_q"]:
                    # x~ = x^T * q ; a_sum column = rsum
                    q_bx = qsc[:].unsqueeze(-1).broadcast_to((128, tpg, 128))
                    nc.vector.tensor_mul(
                        xt_3d[:, :, 0:128], mm1_3d[:, :, 0:128], q_bx)
                    nc.vector.tensor_copy(xt_3d[:, :, 128:129],
                                          rsum[:].unsqueeze(-1))
                else:
                    if xt_dve > 0:
                        nc.vector.tensor_copy(
                            xt_3d[:, :, 0:xt_dve], mm1_3d[:, :, 0:xt_dve])
                    if xt_dve < 128:
                        nc.scalar.copy(
                            xt_3d[:, :, xt_dve:128], mm1_3d[:, :, xt_dve:128])
                    nc.vector.tensor_copy(xt_3d[:, :, 128:129],
                                          normv[:].unsqueeze(-1))

            mm2_tiles = range(tpg)
            if "mm2" in ab:
                mm2_tiles = range(1) if (g == 0 or g == n_groups - 1) else range(0)
            for t in mm2_tiles:
                first = (g == 0 and t == 0)
                last = ("mm2" in ab and g == n_groups - 1 and t == 0) or \
                       (g == n_groups - 1 and t == tpg - 1)
                nc.tensor.matmul(
                    vlad_psum[:, 0:129],
                    lhsT=wt[:, t * K:(t + 1) * K],
                    rhs=xt[:, t * 132: t * 132 + 129],
                    start=first, stop=last, skip_group_check=True,
                )

        # -------- epilogue for sample n --------
        acs = ep_pool.tile([K, C], F32, tag="acs")
        nc.vector.tensor_scalar_mul(acs[:], cent_sb[:], vlad_psum[:, 128:129])
        v = ep_pool.tile([K, C], F32, tag="v")
        nc.vector.tensor_sub(v[:], vlad_psum[:, 0:128], acs[:])
        v2 = ep_pool.tile([K, C], F32, tag="v2")
        nc.vector.tensor_mul(v2[:], v[:], v[:])
        ssqv = sm_pool.tile([K, 1], F32, tag="ssqv")
        nc.vector.tensor_reduce(ssqv[:], v2[:], axis=AX.X, op=ALU.add)
        nv = sm_pool.tile([K, 1], F32, tag="nv")
        nc.scalar.activation(nv[:], ssqv[:], func=AF.Sqrt)
        rnv = sm_pool.tile([K, 1], F32, tag="rnv")
        nc.vector.reciprocal(rnv[:], nv[:])
        rnv8 = sm_pool.tile([K, 1], F32, tag="rnv8")
        nc.vector.tensor_scalar_mul(rnv8[:], rnv[:], 0.125)
        o_t = ep_pool.tile([K, C], F32, tag="o")
        nc.vector.tensor_scalar_mul(o_t[:], v[:], rnv8[:])
        nc.sync.dma_start(out=out_ap[n], in_=o_t[:])


def kernel(x, fc_w, fc_b, centroids):
    """Full-input entry point: shards over 8 cores, returns [N, K*C] float32."""
    from concourse.bass_utils import run_bass_kernel_spmd

    x = np.ascontiguousarray(np.asarray(x, dtype=np.float32))
    fc_w = np.ascontiguousarray(np.asarray(fc_w, dtype=np.float32))
    fc_b = np.ascontiguousarray(np.asarray(fc_b, dtype=np.float32)).reshape(1, K)
    centroids = np.ascontiguousarray(np.asarray(centroids, dtype=np.float32))

    nc = build_nc(N_PER_CORE, S)
    core_ids = list(range(N_CORES))
    in_maps = []
    for i in core_ids:
        shard = x[i * N_PER_CORE:(i + 1) * N_PER_CORE]
        in_maps.append({
            "x": shard,
            "fc_w": fc_w,
            "fc_b": fc_b,
            "centroids": centroids,
        })
    # Retry transient device failures (a crashed tenant can leave the cores
    # "unrecoverable" for a minute or two; they come back on their own).
    last_exc = None
    for attempt in range(4):
        try:
            res = run_bass_kernel_spmd(nc, in_maps, core_ids)
            break
        except Exception as e:  # noqa: BLE001
            last_exc = e
            if attempt == 3:
                raise
            import time as _time
            _time.sleep(45)
    outs = [res.results[i]["out"].reshape(N_PER_CORE, K * C) for i in range(N_CORES)]
    return np.concatenate(outs, axis=0)

